# revision 57
# baseline (speedup 1.0000x reference)
"""nn_AttentionOpt on 8 Trainium2 NeuronCores.

Data-parallel over batch N=8: one batch element per core. Per core
(C=1024 channels, L=1024 positions, H=16 heads, dh=64):

    x = seq^T                        (L, C)
    Q/K = relu(x @ Wq^T + b)         fp8e4, score-sharded layout (see below)
    V   = relu(x @ Wv^T + bv) * m    fp16, augmented with the key mask as
                                     64 extra rows (softmax denominator)
    S^T = K_h Q_h^T                  fp8 DoubleRow matmuls ([k, q] layout)
    E   = exp(S^T / (8*SW^2))        ACT, written as fp16
    P   = [m*ones | V_h]^T E^T       fp16 matmuls: rows 0:64 = denominator,
                                     rows 64:128 = unnormalized out^T
    y_h = P[64:128] * approx_recip(P[0:64])      (fp16, kept for LN stats)
    LayerNorm over C with one-pass stats (sum y and sum y^2 reduced over
    partitions by ones-matmuls during attention), post-mask only.

Performance structure (cost-model driven):
  * Projections run as fp8e4 DoubleRow matmuls (0.5 cycles/row, double
    contraction) with an error-compensating split: x = x_hi + x_lo and
    W = W_hi + W_lo in fp8, computing hi*hi + lo*hi + hi*lo (~bf16-level
    accuracy at ~2.7x the bf16 matmul rate).
  * Weights are staged host-side: pre-transposed, pre-scaled by SW=32 (to
    keep fp8 values in the normal range), pre-split hi/lo, and (for Wq/Wk)
    row-permuted so the projection PSUM tiles land directly in the
    [32-channel, 2-ksubtile] layout DoubleRow score matmuls need.
  * Scores use single fp8 Q/K (the dominant error term, ~1.2e-2 of the
    2e-2 budget); the attention-value matmul uses fp16 E and V (error
    ~3e-4, same PE cost as bf16).
  * The exp stream on ACT (~128 x [128,1024] activations, ~135us) and the
    PE matmul stream (~155us) are co-bottlenecks; PE work is interleaved
    behind the score stream via a background queue.

Scale bookkeeping: Q,K,V all carry SW=32 -> scores carry SW^2 (folded into
the exp scale), y carries SW (cancelled by LayerNorm; EPS scaled by SW^2).
"""
import sys

if "/opt/trn_rl_repo" not in sys.path:
    sys.path.insert(0, "/opt/trn_rl_repo")

from collections import deque
from contextlib import ExitStack

import numpy as np

import concourse.bass as bass
import concourse.tile as tile
from concourse import bacc, mybir
from concourse.bass_utils import run_bass_kernel_spmd

f32 = mybir.dt.float32
f32r = mybir.dt.float32r
bf16 = mybir.dt.bfloat16
f16 = mybir.dt.float16
f8 = mybir.dt.float8e4
AF = mybir.ActivationFunctionType
ALU = mybir.AluOpType
DR = mybir.MatmulPerfMode.DoubleRow

N_CORES = 8
C = 1024
L = 1024
H = 16
DH = 64
P = 128
NCH = C // P          # 8 chunks of channels
NLC = L // P          # 8 chunks of positions (key chunks)
FD = 512              # matmul moving free dim (one PSUM bank of f32)
NQ = L // FD          # 2 query halves
SW = 32.0             # host-side weight scale (fp8 normal range)
EPS = 1e-5 * SW * SW  # LN epsilon in the SW-scaled domain
SCALE = 1.0 / (8.0 * SW * SW)   # 1/sqrt(dh) folded with 1/SW^2
CT = 32.0             # Q/K centering offset (SW-scaled; = 1.0 unscaled).
                      # Q/K are stored as fp8(value - CT), halving their
                      # quantization error; the per-key correction folds into
                      # the exp bias, the per-query part cancels in softmax.

USE_FP8_SCORES = True

_BUILT = {}
LAST_RESULTS = None


def _bcast_ap(ap, n):
    """Prepend a [0, n] partition-broadcast dim to an AP (DRAM source)."""
    return bass.AP(tensor=ap.tensor, offset=ap.offset, ap=[[0, n]] + list(ap.ap))


def _emit(tc, io, fast_ln):
    nc = tc.nc
    out = io["out"]

    with ExitStack() as ctx:
        persist = ctx.enter_context(tc.tile_pool(name="persist", bufs=1))
        dramp = ctx.enter_context(tc.tile_pool(name="dram", bufs=1,
                                               space="DRAM"))

        # ---- constants / small inputs ------------------------------------
        bq_sb = persist.tile([P, NCH], f32, tag="bq")
        bk_sb = persist.tile([P, NCH], f32, tag="bk")
        nc.gpsimd.dma_start(out=bq_sb[:], in_=io["bq"].rearrange("(a p) -> p a", p=P))
        nc.gpsimd.dma_start(out=bk_sb[:], in_=io["bk"].rearrange("(a p) -> p a", p=P))
        m_l = persist.tile([P, NLC], f32, tag="ml")
        nc.gpsimd.dma_start(out=m_l[:], in_=io["maskf"].rearrange("(a p) -> p a", p=P))
        bv_row = persist.tile([1, C], bf16, tag="bvrow")
        nc.gpsimd.dma_start(out=bv_row[:], in_=io["bv"].rearrange("(a c) -> a c", a=1))
        lnw_sb = persist.tile([P, NCH], f32, tag="lnw")
        lnb_sb = persist.tile([P, NCH], f32, tag="lnb")
        if not fast_ln:
            nc.gpsimd.dma_start(out=lnw_sb[:],
                                in_=io["ln_w"].rearrange("(a p) -> p a", p=P))
            nc.gpsimd.dma_start(out=lnb_sb[:],
                                in_=io["ln_b"].rearrange("(a p) -> p a", p=P))
            m_rep = persist.tile([P, L], f32, tag="mrep")
            nc.gpsimd.dma_start(out=m_rep[:], in_=_bcast_ap(io["maskf"], P))

        ones_f = persist.tile([P, 1], f32, tag="onesf")
        nc.vector.memset(ones_f[:], 1.0)
        ones_colh = persist.tile([P, 1], f16, tag="onesh")
        nc.vector.tensor_copy(ones_colh[:], ones_f[:])
        zero_col = persist.tile([P, 1], f32, tag="zero")
        nc.vector.memset(zero_col[:], 0.0)
        negc_col = persist.tile([P, 1], f32, tag="negc")
        nc.vector.memset(negc_col[:], -CT)
        # stationary column for the per-key score-centering correction:
        # bias = CT * sum_d k' / 8 = sum_d (CT*SCALE) * k~'  (k~' = SW*k')
        cones = persist.tile([P, 2, 1], f8, tag="cones")
        nc.vector.memset(cones[:], CT * SCALE)
        # per-head, per-key-chunk exp bias columns (+ DRAM bounce row for
        # the PSUM-row -> SBUF-column rearrange)
        sbias = persist.tile([P, H, NLC], f32, tag="sbias")
        sb_scr = dramp.tile([H, L], f32, tag="sbscr")
        ones1 = persist.tile([1, P], bf16, tag="ones1")
        nc.vector.memset(ones1[:], 1.0)
        # ones rows at partitions 0 and 32 for row-broadcast matmuls
        ones_bank = persist.tile([33, P], f16, tag="onesbank")
        nc.vector.memset(ones_bank[:], 1.0)

        # packed row storage. Constraints: engine access base partition must
        # be 0/32/64, and dual-SBUF-input ops need equal input bases.
        # rowsF1 (f32): 0 sum_y (later rstd), 32 sum_y2
        # rowsF2 (f32): 0 u then var, 32 u^2
        # rowsF3 (f32): 0 mask row, 32 ln(var+eps)
        # rowsMM (f16): 0 u (matmul rhs), 32 rstd*m or rstd (matmul rhs)
        # sb_row (f32): PSUM->SBUF staging for the score-centering bias rows
        rowsF1 = persist.tile([33, L], f32, tag="rowsF1")
        rowsF2 = persist.tile([33, L], f32, tag="rowsF2")
        rowsF3 = persist.tile([33, L], f32, tag="rowsF3")
        rowsMM = persist.tile([33, L], f16, tag="rowsMM")
        sb_row = persist.tile([1, L], f32, tag="sbrow")
        nc.vector.memset(rowsF1[32:33, :], 0.0)
        nc.gpsimd.dma_start(
            out=rowsF3[0:1, :], in_=io["maskf"].rearrange("(a l) -> a l", a=1))

        # ---- big persistent tensors --------------------------------------
        if USE_FP8_SCORES:
            # [32*(h%4)+d%32, h//4, d//32, pos] per head h, head-channel d
            qT = persist.tile([P, 4, 2, L], f8, tag="qT")
            kT = persist.tile([P, 4, 2, L], f8, tag="kT")
        else:
            qT = persist.tile([P, NCH, L], bf16, tag="qT")
            kT = persist.tile([P, NCH, L], bf16, tag="kT")
        v_aug = persist.tile([P, NLC, H, P], f16, tag="vaug")
        y_sb = persist.tile([P, NCH, L], f16, tag="y")

        with tc.tile_pool(name="wts", bufs=1) as wts, \
             tc.tile_pool(name="mm", bufs=2, space="PSUM") as mmp, \
             tc.tile_pool(name="scp", bufs=2, space="PSUM") as scp, \
             tc.tile_pool(name="stats", bufs=1, space="PSUM") as statsp, \
             tc.tile_pool(name="eT", bufs=3) as epool, \
             tc.tile_pool(name="att", bufs=2) as attp, \
             tc.tile_pool(name="sq", bufs=1) as sqp:

            ps_sy = statsp.tile([1, L], f32, tag="sy")

            seq_hi = wts.tile([P, NCH, L], f8, tag="seqh")
            seq_lo = wts.tile([P, NCH, L], f8, tag="seql")
            w_sb = {}
            for name in ("wq", "wk", "wv"):
                for half in ("hi", "lo"):
                    w_sb[name, half] = wts.tile([P, NCH, C], f8,
                                                tag=f"{name}{half}",
                                                name=f"{name}{half}")
            # DMA order = DMA_ENGINES serialization order: critical path
            # first. The prologue (Q/K chunks 0-1) needs seq hi+lo and W
            # cols 0:256 of wq/wk hi+lo; stream those, then remainders.
            B0 = 2 * P

            def w_dma(eng, name, half, lo, hi):
                eng.dma_start(
                    out=w_sb[name, half][:, :, lo:hi],
                    in_=io[f"{name}_{half}"][:, lo:hi].rearrange(
                        "(cc p) t -> p cc t", p=P))

            nc.sync.dma_start(
                out=seq_hi[:],
                in_=io["seq_hi"].rearrange("(cc p) l -> p cc l", p=P))
            w_dma(nc.scalar, "wk", "hi", 0, B0)
            nc.sync.dma_start(
                out=seq_lo[:],
                in_=io["seq_lo"].rearrange("(cc p) l -> p cc l", p=P))
            w_dma(nc.scalar, "wk", "lo", 0, B0)
            w_dma(nc.scalar, "wq", "hi", 0, B0)
            w_dma(nc.scalar, "wq", "lo", 0, B0)
            # wv cols 0:512 feed projv(0) in head slots 0-1: before the big
            # wq/wk remainders (needed only from slot 2 on)
            w_dma(nc.sync, "wv", "hi", 0, FD)
            w_dma(nc.sync, "wv", "lo", 0, FD)
            w_dma(nc.sync, "wk", "hi", B0, C)
            w_dma(nc.sync, "wk", "lo", B0, C)
            w_dma(nc.scalar, "wq", "hi", B0, C)
            w_dma(nc.scalar, "wq", "lo", B0, C)
            w_dma(nc.sync, "wv", "hi", FD, C)
            w_dma(nc.sync, "wv", "lo", FD, C)

            # ---- emission helpers ----------------------------------------
            PROJ_TERMS = (("hi", "hi"), ("lo", "hi"), ("hi", "lo"))

            def qk_lh(wname, dc, lh):
                """One [128, 512] projection tile of Q or K."""
                tgt = qT if wname == "wq" else kT
                b_sb = bq_sb if wname == "wq" else bk_sb
                xs = {"hi": seq_hi, "lo": seq_lo}
                ps = mmp.tile([P, FD], f32, tag="mm")
                n = len(PROJ_TERMS) * 4
                i = 0
                for wh, xh in PROJ_TERMS:
                    wt = w_sb[wname, wh]
                    xt = xs[xh]
                    for ccp in range(4):
                        nc.tensor.matmul(
                            ps[:],
                            lhsT=wt[:, 2 * ccp:2 * ccp + 2, dc * P:(dc + 1) * P],
                            rhs=xt[:, 2 * ccp:2 * ccp + 2, lh * FD:(lh + 1) * FD],
                            start=(i == 0), stop=(i == n - 1), perf_mode=DR)
                        i += 1
                if USE_FP8_SCORES:
                    # centered: max(z + b - CT, -CT) = relu(z + b) - CT
                    # (host pre-subtracts CT from the bias)
                    dst = tgt[:, dc // 2, dc % 2, lh * FD:(lh + 1) * FD]
                    lim = negc_col
                else:
                    dst = tgt[:, dc, lh * FD:(lh + 1) * FD]
                    lim = zero_col
                nc.vector.scalar_tensor_tensor(
                    out=dst, in0=ps[:], scalar=b_sb[:, dc:dc + 1],
                    in1=lim[:].to_broadcast((P, FD)),
                    op0=ALU.add, op1=ALU.max)

            def projv_lc(dh2, lc):
                """One [128 positions, 512 channels] V tile -> v_aug."""
                xs = {"hi": seq_hi, "lo": seq_lo}
                ps = mmp.tile([P, FD], f32, tag="mm")
                nc.tensor.matmul(
                    ps[:], lhsT=ones1[:], rhs=bv_row[:, dh2 * FD:(dh2 + 1) * FD],
                    start=True, stop=False)
                n = len(PROJ_TERMS) * 4
                i = 0
                for wh, xh in PROJ_TERMS:
                    wt = w_sb["wv", wh]
                    xt = xs[xh]
                    for ccp in range(4):
                        nc.tensor.matmul(
                            ps[:],
                            lhsT=xt[:, 2 * ccp:2 * ccp + 2, lc * P:(lc + 1) * P],
                            rhs=wt[:, 2 * ccp:2 * ccp + 2, dh2 * FD:(dh2 + 1) * FD],
                            start=False, stop=(i == n - 1), perf_mode=DR)
                        i += 1
                # relu + key-position mask: (ps max 0) * m[l]
                nc.vector.scalar_tensor_tensor(
                    out=v_aug[:, lc, dh2 * (H // 2):(dh2 + 1) * (H // 2), DH:P],
                    in0=ps[:], scalar=0.0,
                    in1=m_l[:, lc:lc + 1].to_broadcast((P, FD)),
                    op0=ALU.max, op1=ALU.mult)

            def av_qh(h, eT, qh):
                """Attention-value matmul + normalization for one query half."""
                ps = mmp.tile([P, FD], f32, tag="mm")
                for kc in range(NLC):
                    nc.tensor.matmul(
                        ps[:],
                        lhsT=v_aug[:, kc, h, :],
                        rhs=eT[:, kc, qh * FD:(qh + 1) * FD],
                        start=(kc == 0), stop=(kc == NLC - 1))
                rcp = attp.tile([DH, FD], f32, tag="rcp")
                nc.vector.reciprocal_approx_fast(out=rcp[:], in_=ps[0:DH, :])
                off = DH * (h % 2)
                nc.vector.tensor_mul(
                    y_sb[off:off + DH, h // 2, qh * FD:(qh + 1) * FD],
                    ps[DH:P, :], rcp[:])

            def stats_cc(cc):
                """Reduce sum_c y (PSUM accumulator) and sum_c y^2 (transient
                mm-tile row + SBUF row-add; PSUM banks are fully booked)."""
                ysq = sqp.tile([P, L], f16, tag="ysq")
                nc.gpsimd.tensor_mul(ysq[:], y_sb[:, cc, :], y_sb[:, cc, :])
                for qh in range(NQ):
                    nc.tensor.matmul(
                        ps_sy[0:1, qh * FD:(qh + 1) * FD],
                        lhsT=ones_colh[:],
                        rhs=y_sb[:, cc, qh * FD:(qh + 1) * FD],
                        start=(cc == 0), stop=(cc == NCH - 1))
                    st = mmp.tile([P, FD], f32, tag="mm",
                                  name=f"st_{cc}_{qh}")
                    nc.tensor.matmul(st[0:1, :], lhsT=ones_colh[:],
                                     rhs=ysq[:, qh * FD:(qh + 1) * FD],
                                     start=True, stop=True)
                    nc.vector.tensor_add(
                        rowsF1[32:33, qh * FD:(qh + 1) * FD],
                        rowsF1[32:33, qh * FD:(qh + 1) * FD], st[0:1, :])

            bg = deque()

            def bg_run(n):
                for _ in range(n):
                    if not bg:
                        return
                    bg.popleft()()

            eTs = {}

            def bias_prep(h):
                # per-key centering correction -> exp bias columns:
                # row = sum_d (CT*SCALE)*k~' over the head's channels, bounced
                # through DRAM to turn the key-row into per-chunk columns.
                g, r = h // 4, h % 4
                for half in range(NQ):
                    st = mmp.tile([P, FD], f32, tag="mm",
                                  name=f"sb_{h}_{half}")
                    nc.tensor.matmul(
                        st[0:1, :],
                        lhsT=cones[32 * r:32 * r + 32, :, 0:1],
                        rhs=kT[32 * r:32 * r + 32, g, 0:2,
                               half * FD:(half + 1) * FD],
                        start=True, stop=True, perf_mode=DR,
                        tile_position=(32 * r, 0))
                    nc.vector.tensor_copy(
                        sb_row[0:1, half * FD:(half + 1) * FD], st[0:1, :])
                    nc.sync.dma_start(
                        out=sb_scr[h, half * FD:(half + 1) * FD],
                        in_=sb_row[0:1, half * FD:(half + 1) * FD])
                    nc.sync.dma_start(
                        out=sbias[:, h, half * 4:(half + 1) * 4],
                        in_=sb_scr[h, half * FD:(half + 1) * FD].rearrange(
                            "(kc p) -> p kc", p=P))

            def scores_head(h):
                eT = epool.tile([P, NLC, L], f16, tag="eT", name=f"eT{h}")
                eTs[h] = eT
                g, r = h // 4, h % 4
                hp, off = h // 2, DH * (h % 2)
                for kc in range(NLC):
                    ps = scp.tile([P, L], f32, tag="sc")
                    for qh in range(NQ):
                        if USE_FP8_SCORES:
                            nc.tensor.matmul(
                                ps[:, qh * FD:(qh + 1) * FD],
                                lhsT=kT[32 * r:32 * r + 32, g, 0:2,
                                        kc * P:(kc + 1) * P],
                                rhs=qT[32 * r:32 * r + 32, g, 0:2,
                                       qh * FD:(qh + 1) * FD],
                                start=True, stop=True, perf_mode=DR,
                                tile_position=(32 * r, 0))
                        else:
                            nc.tensor.matmul(
                                ps[:, qh * FD:(qh + 1) * FD],
                                lhsT=kT[off:off + DH, hp, kc * P:(kc + 1) * P],
                                rhs=qT[off:off + DH, hp, qh * FD:(qh + 1) * FD],
                                start=True, stop=True)
                    if USE_FP8_SCORES:
                        nc.scalar.activation(eT[:, kc, :], ps[:], AF.Exp,
                                             scale=SCALE,
                                             bias=sbias[:, h, kc:kc + 1])
                    else:
                        nc.scalar.activation(eT[:, kc, :], ps[:], AF.Exp,
                                             scale=SCALE)
                    if kc >= 1:
                        bg_run(1)
                if USE_FP8_SCORES and h + 1 < H:
                    bias_prep(h + 1)

            def qk_items(dc):
                return [lambda w=w, dc=dc, lh=lh: qk_lh(w, dc, lh)
                        for w in ("wk", "wq") for lh in range(NQ)]

            def av_items(h):
                return [lambda h=h, qh=qh: av_qh(h, eTs[h], qh)
                        for qh in range(NQ)]

            # ---- prologue: Q/K chunks 0,1 feed the first head group ------
            for dc in (0, 1):
                for it in qk_items(dc):
                    it()
            if USE_FP8_SCORES:
                bias_prep(0)
            # mask columns 0:64 of v_aug (denominator rows of the AV matmul)
            ml_b = bass.AP(tensor=m_l.tensor, offset=m_l.offset,
                           ap=list(m_l[:].ap[:2]) + [[0, H], [0, DH]])
            nc.gpsimd.tensor_copy(v_aug[:, :, :, 0:DH], ml_b)

            # background enqueue plan, keyed by head slot
            plan = {
                0: [lambda lc=lc: projv_lc(0, lc) for lc in range(4)],
                1: [lambda lc=lc: projv_lc(0, lc) for lc in range(4, 8)],
                2: av_items(0) + qk_items(2),
                3: av_items(1) + [lambda: stats_cc(0)] + qk_items(3),
                4: av_items(2),
                5: av_items(3) + [lambda: stats_cc(1)] + qk_items(4),
                6: av_items(4) + qk_items(5),
                7: av_items(5) + [lambda: stats_cc(2)]
                   + [lambda lc=lc: projv_lc(1, lc) for lc in range(4)],
                8: av_items(6)
                   + [lambda lc=lc: projv_lc(1, lc) for lc in range(4, 8)],
                9: av_items(7) + [lambda: stats_cc(3)] + qk_items(6),
                10: av_items(8) + qk_items(7),
                11: av_items(9) + [lambda: stats_cc(4)],
                12: av_items(10),
                13: av_items(11) + [lambda: stats_cc(5)],
                14: av_items(12),
                15: av_items(13) + [lambda: stats_cc(6)] + av_items(14),
            }
            for h in range(H):
                if h in plan:
                    bg.extend(plan[h])
                scores_head(h)
            while bg:
                bg.popleft()()
            for it in av_items(15):
                it()
            stats_cc(7)
            # u = sum_y / C (read the PSUM accumulator before scope close)
            nc.vector.tensor_scalar_mul(rowsF2[0:1, :], ps_sy[0:1, :], 1.0 / C)

        # ---- LayerNorm tail ----------------------------------------------
        # var = sum_y2 / C - u^2  (one-pass stats; u^2 from the f32 u --
        # fp16 u quantization would wreck var via ~100x cancellation)
        nc.vector.tensor_copy(rowsMM[0:1, :], rowsF2[0:1, :])
        nc.gpsimd.tensor_mul(rowsF2[32:33, :], rowsF2[0:1, :], rowsF2[0:1, :])
        nc.vector.scalar_tensor_tensor(
            out=rowsF2[0:1, :], in0=rowsF1[32:33, :], scalar=1.0 / C,
            in1=rowsF2[32:33, :], op0=ALU.mult, op1=ALU.subtract)
        eps_col = persist.tile([1, 1], f32, tag="eps")
        nc.vector.memset(eps_col[:], EPS)
        nc.scalar.activation(rowsF3[32:33, :], rowsF2[0:1, :], AF.Ln,
                             bias=eps_col[:, 0:1])
        nc.scalar.activation(rowsF1[0:1, :], rowsF3[32:33, :], AF.Exp,
                             scale=-0.5)

        with tc.tile_pool(name="tailps", bufs=1, space="PSUM") as tailps, \
             tc.tile_pool(name="norm", bufs=2) as norm:
            u_rep = tailps.tile([P, L], f32, tag="urep")
            for qh in range(NQ):
                nc.tensor.matmul(u_rep[:, qh * FD:(qh + 1) * FD],
                                 lhsT=ones_bank[0:1, :],
                                 rhs=rowsMM[0:1, qh * FD:(qh + 1) * FD],
                                 start=True, stop=True)
            u_rep_sb = norm.tile([P, L], f16, tag="ureps", bufs=1)
            nc.vector.tensor_copy(u_rep_sb[:], u_rep[:])

            out_r = out.rearrange("(cc p) l -> p cc l", p=P)
            if fast_ln:
                # ln_w == 1, ln_b == 0: out = (y - u) * (rstd * m), with the
                # f16 -> f32 widening done by the output DMA (SWDGE cast).
                nc.gpsimd.tensor_mul(rowsMM[32:33, :], rowsF1[0:1, :],
                                     rowsF3[0:1, :])
                rm_rep = tailps.tile([P, L], f32, tag="rmrep")
                for qh in range(NQ):
                    nc.tensor.matmul(rm_rep[:, qh * FD:(qh + 1) * FD],
                                     lhsT=ones_bank[32:33, :],
                                     rhs=rowsMM[32:33, qh * FD:(qh + 1) * FD],
                                     start=True, stop=True)
                rm_rep_sb = norm.tile([P, L], f16, tag="rmreps", bufs=1)
                nc.vector.tensor_copy(rm_rep_sb[:], rm_rep[:])
                for cc in range(NCH):
                    s = norm.tile([P, L], f16, tag="s", name=f"s_{cc}")
                    nc.vector.tensor_sub(s[:], y_sb[:, cc, :], u_rep_sb[:])
                    t3 = norm.tile([P, L], f16, tag="t3", name=f"t3_{cc}")
                    nc.vector.tensor_mul(t3[:], s[:], rm_rep_sb[:])
                    nc.gpsimd.dma_start(out=out_r[:, cc, :], in_=t3[:])
            else:
                nc.vector.tensor_copy(rowsMM[32:33, :], rowsF1[0:1, :])
                rm_rep = tailps.tile([P, L], f32, tag="rmrep")
                for qh in range(NQ):
                    nc.tensor.matmul(rm_rep[:, qh * FD:(qh + 1) * FD],
                                     lhsT=ones_bank[32:33, :],
                                     rhs=rowsMM[32:33, qh * FD:(qh + 1) * FD],
                                     start=True, stop=True)
                rm_rep_sb = norm.tile([P, L], f32, tag="rmrepsf", bufs=1)
                nc.vector.tensor_copy(rm_rep_sb[:], rm_rep[:])
                for cc in range(NCH):
                    t1 = norm.tile([P, L], f32, tag="t1", name=f"t1_{cc}")
                    nc.vector.tensor_sub(t1[:], y_sb[:, cc, :], u_rep_sb[:])
                    t2 = norm.tile([P, L], f32, tag="t2", name=f"t2_{cc}")
                    nc.vector.scalar_tensor_tensor(
                        out=t2[:], in0=t1[:], scalar=lnw_sb[:, cc:cc + 1],
                        in1=rm_rep_sb[:], op0=ALU.mult, op1=ALU.mult)
                    t3 = norm.tile([P, L], f32, tag="t3", name=f"t3_{cc}")
                    nc.vector.scalar_tensor_tensor(
                        out=t3[:], in0=t2[:], scalar=lnb_sb[:, cc:cc + 1],
                        in1=m_rep[:], op0=ALU.add, op1=ALU.mult)
                    dmae = nc.sync if cc % 2 == 0 else nc.scalar
                    dmae.dma_start(out=out_r[:, cc, :], in_=t3[:])


def _pin_act_table(nc):
    """Make every activation resolve to the one table that contains all the
    functions this kernel uses (Exp, Ln, Copy, Identity), so the compiler
    emits a single LoadActFuncSet."""
    from concourse.hw_specs import get_activation_tables
    keep = "natural_log_exp_and_others"
    try:
        tabs = get_activation_tables(nc.m.arch)
    except Exception:
        return
    if keep not in tabs:
        return
    shared = set(tabs[keep])
    for name, funcs in tabs.items():
        if name != keep:
            funcs -= shared


def build(fast_ln=True):
    if fast_ln in _BUILT:
        return _BUILT[fast_ln]
    nc = bacc.Bacc("TRN2", target_bir_lowering=False, debug=False,
                   num_devices=N_CORES)
    _pin_act_table(nc)
    io = {
        "seq_hi": nc.dram_tensor("seq_hi", [C, L], f8, kind="ExternalInput").ap(),
        "seq_lo": nc.dram_tensor("seq_lo", [C, L], f8, kind="ExternalInput").ap(),
        "maskf": nc.dram_tensor("maskf", [L], f32, kind="ExternalInput").ap(),
        "bq": nc.dram_tensor("bq", [C], f32, kind="ExternalInput").ap(),
        "bk": nc.dram_tensor("bk", [C], f32, kind="ExternalInput").ap(),
        "bv": nc.dram_tensor("bv", [C], bf16, kind="ExternalInput").ap(),
        "ln_w": nc.dram_tensor("ln_w", [C], f32, kind="ExternalInput").ap(),
        "ln_b": nc.dram_tensor("ln_b", [C], f32, kind="ExternalInput").ap(),
        "out": nc.dram_tensor("out", [C, L], f32, kind="ExternalOutput").ap(),
    }
    for name in ("wq", "wk", "wv"):
        for half in ("hi", "lo"):
            t = f"{name}_{half}"
            io[t] = nc.dram_tensor(t, [C, C], f8, kind="ExternalInput").ap()
    with tile.TileContext(nc) as tc:
        _emit(tc, io, fast_ln)
    nc.compile()
    _BUILT[fast_ln] = nc
    return nc


def _qk_perm():
    """Permutation of W rows so projection PSUM tiles land in the DoubleRow
    score layout: slot (tile tau, partition j) <- channel 64*h + d with
    h = 4*(tau//2) + j//32, d = 32*(tau%2) + j%32."""
    if not USE_FP8_SCORES:
        return np.arange(C)
    perm = np.empty(C, dtype=np.int64)
    for tau in range(NCH):
        for j in range(P):
            h = 4 * (tau // 2) + j // 32
            d = 32 * (tau % 2) + j % 32
            perm[tau * P + j] = 64 * h + d
    return perm


def _split_fp8(a):
    import ml_dtypes
    hi = a.astype(ml_dtypes.float8_e4m3)
    lo = (a - hi.astype(np.float32)).astype(ml_dtypes.float8_e4m3)
    return np.ascontiguousarray(hi), np.ascontiguousarray(lo)


def make_in_maps(seq, mask, wq, bq, wk, bk, wv, bv, ln_w, ln_b):
    import ml_dtypes
    seq = np.asarray(seq, dtype=np.float32)
    mask_f = np.ascontiguousarray(
        np.asarray(mask).astype(np.float32).reshape(N_CORES, L))
    perm = _qk_perm()
    wq_hi, wq_lo = _split_fp8(
        np.asarray(wq, np.float32)[perm, :].T * SW)
    wk_hi, wk_lo = _split_fp8(
        np.asarray(wk, np.float32)[perm, :].T * SW)
    wv_hi, wv_lo = _split_fp8(np.asarray(wv, np.float32).T * SW)
    shared = {
        "wq_hi": wq_hi, "wq_lo": wq_lo,
        "wk_hi": wk_hi, "wk_lo": wk_lo,
        "wv_hi": wv_hi, "wv_lo": wv_lo,
        "bq": np.ascontiguousarray(
            np.asarray(bq, np.float32)[perm] * SW
            - (CT if USE_FP8_SCORES else 0.0)),
        "bk": np.ascontiguousarray(
            np.asarray(bk, np.float32)[perm] * SW
            - (CT if USE_FP8_SCORES else 0.0)),
        "bv": np.ascontiguousarray(
            (np.asarray(bv, np.float32) * SW).astype(ml_dtypes.bfloat16)),
        "ln_w": np.ascontiguousarray(np.asarray(ln_w, dtype=np.float32)),
        "ln_b": np.ascontiguousarray(np.asarray(ln_b, dtype=np.float32)),
    }
    maps = []
    for i in range(N_CORES):
        s_hi, s_lo = _split_fp8(seq[i])
        maps.append({"seq_hi": s_hi, "seq_lo": s_lo, "maskf": mask_f[i],
                     **shared})
    return maps


def kernel(seq, mask, wq, bq, wk, bk, wv, bv, ln_w, ln_b):
    global LAST_RESULTS
    fast_ln = bool(np.all(np.asarray(ln_w) == 1.0)
                   and np.all(np.asarray(ln_b) == 0.0))
    nc = build(fast_ln)
    in_maps = make_in_maps(seq, mask, wq, bq, wk, bk, wv, bv, ln_w, ln_b)
    res = run_bass_kernel_spmd(nc, in_maps, list(range(N_CORES)))
    LAST_RESULTS = res
    return np.stack([res.results[i]["out"] for i in range(N_CORES)], axis=0)


# revision 59
# speedup vs baseline: 1.1350x; 1.1350x over previous
"""nn_AttentionOpt on 8 Trainium2 NeuronCores.

Data-parallel over batch N=8: one batch element per core. Per core
(C=1024 channels, L=1024 positions, H=16 heads, dh=64):

    x = seq^T                        (L, C)
    Q/K = relu(x @ Wq^T + b)         fp8e4, score-sharded layout (see below)
    V   = relu(x @ Wv^T + bv) * m    fp16, augmented with the key mask as
                                     64 extra rows (softmax denominator)
    S^T = K_h Q_h^T                  fp8 DoubleRow matmuls ([k, q] layout)
    E   = exp(S^T / (8*SW^2))        ACT, written as fp16
    P   = [m*ones | V_h]^T E^T       fp16 matmuls: rows 0:64 = denominator,
                                     rows 64:128 = unnormalized out^T
    y_h = P[64:128] * approx_recip(P[0:64])      (fp16, kept for LN stats)
    LayerNorm over C with one-pass stats (sum y and sum y^2 reduced over
    partitions by ones-matmuls during attention), post-mask only.

Performance structure (cost-model driven):
  * Projections run as fp8e4 DoubleRow matmuls (0.5 cycles/row, double
    contraction) with an error-compensating split: x = x_hi + x_lo and
    W = W_hi + W_lo in fp8, computing hi*hi + lo*hi + hi*lo (~bf16-level
    accuracy at ~2.7x the bf16 matmul rate).
  * Weights are staged host-side: pre-transposed, pre-scaled by SW=32 (to
    keep fp8 values in the normal range), pre-split hi/lo, and (for Wq/Wk)
    row-permuted so the projection PSUM tiles land directly in the
    [32-channel, 2-ksubtile] layout DoubleRow score matmuls need.
  * Scores use single fp8 Q/K (the dominant error term, ~1.2e-2 of the
    2e-2 budget); the attention-value matmul uses fp16 E and V (error
    ~3e-4, same PE cost as bf16).
  * The exp stream on ACT (~128 x [128,1024] activations, ~135us) and the
    PE matmul stream (~155us) are co-bottlenecks; PE work is interleaved
    behind the score stream via a background queue.

Scale bookkeeping: Q,K,V all carry SW=32 -> scores carry SW^2 (folded into
the exp scale), y carries SW (cancelled by LayerNorm; EPS scaled by SW^2).
"""
import sys

if "/opt/trn_rl_repo" not in sys.path:
    sys.path.insert(0, "/opt/trn_rl_repo")

from collections import deque
from contextlib import ExitStack

import numpy as np

import concourse.bass as bass
import concourse.tile as tile
from concourse import bacc, mybir
from concourse.bass_utils import run_bass_kernel_spmd

f32 = mybir.dt.float32
f32r = mybir.dt.float32r
bf16 = mybir.dt.bfloat16
f16 = mybir.dt.float16
f8 = mybir.dt.float8e4
AF = mybir.ActivationFunctionType
ALU = mybir.AluOpType
DR = mybir.MatmulPerfMode.DoubleRow

N_CORES = 8
C = 1024
L = 1024
H = 16
DH = 64
P = 128
NCH = C // P          # 8 chunks of channels
NLC = L // P          # 8 chunks of positions (key chunks)
FD = 512              # matmul moving free dim (one PSUM bank of f32)
NQ = L // FD          # 2 query halves
SW = 32.0             # host-side weight scale (fp8 normal range)
EPS = 1e-5 * SW * SW  # LN epsilon in the SW-scaled domain
SCALE = 1.0 / (8.0 * SW * SW)   # 1/sqrt(dh) folded with 1/SW^2
CT = 32.0             # Q/K centering offset (SW-scaled; = 1.0 unscaled).
                      # Q/K are stored as fp8(value - CT), halving their
                      # quantization error; the per-key correction folds into
                      # the exp bias, the per-query part cancels in softmax.

USE_FP8_SCORES = True

_BUILT = {}
LAST_RESULTS = None


def _bcast_ap(ap, n):
    """Prepend a [0, n] partition-broadcast dim to an AP (DRAM source)."""
    return bass.AP(tensor=ap.tensor, offset=ap.offset, ap=[[0, n]] + list(ap.ap))


def _emit(tc, io, fast_ln):
    nc = tc.nc
    out = io["out"]

    with ExitStack() as ctx:
        persist = ctx.enter_context(tc.tile_pool(name="persist", bufs=1))
        dramp = ctx.enter_context(tc.tile_pool(name="dram", bufs=1,
                                               space="DRAM"))

        # ---- constants / small inputs ------------------------------------
        bq_sb = persist.tile([P, NCH], f32, tag="bq")
        bk_sb = persist.tile([P, NCH], f32, tag="bk")
        nc.gpsimd.dma_start(out=bq_sb[:], in_=io["bq"].rearrange("(a p) -> p a", p=P))
        nc.gpsimd.dma_start(out=bk_sb[:], in_=io["bk"].rearrange("(a p) -> p a", p=P))
        m_l = persist.tile([P, NLC], f32, tag="ml")
        nc.gpsimd.dma_start(out=m_l[:], in_=io["maskf"].rearrange("(a p) -> p a", p=P))
        bv_row = persist.tile([1, C], bf16, tag="bvrow")
        nc.gpsimd.dma_start(out=bv_row[:], in_=io["bv"].rearrange("(a c) -> a c", a=1))
        lnw_sb = persist.tile([P, NCH], f32, tag="lnw")
        lnb_sb = persist.tile([P, NCH], f32, tag="lnb")
        if not fast_ln:
            nc.gpsimd.dma_start(out=lnw_sb[:],
                                in_=io["ln_w"].rearrange("(a p) -> p a", p=P))
            nc.gpsimd.dma_start(out=lnb_sb[:],
                                in_=io["ln_b"].rearrange("(a p) -> p a", p=P))
            m_rep = persist.tile([P, L], f32, tag="mrep")
            nc.gpsimd.dma_start(out=m_rep[:], in_=_bcast_ap(io["maskf"], P))

        ones_f = persist.tile([P, 1], f32, tag="onesf")
        nc.vector.memset(ones_f[:], 1.0)
        ones_colh = persist.tile([P, 1], f16, tag="onesh")
        nc.vector.tensor_copy(ones_colh[:], ones_f[:])
        zero_col = persist.tile([P, 1], f32, tag="zero")
        nc.vector.memset(zero_col[:], 0.0)
        negc_col = persist.tile([P, 1], f32, tag="negc")
        nc.vector.memset(negc_col[:], -CT)
        # stationary column for the per-key score-centering correction:
        # bias = CT * sum_d k' / 8 = sum_d (CT*SCALE) * k~'  (k~' = SW*k')
        cones = persist.tile([P, 2, 1], f8, tag="cones")
        nc.vector.memset(cones[:], CT * SCALE)
        # per-head, per-key-chunk exp bias columns (+ DRAM bounce row for
        # the PSUM-row -> SBUF-column rearrange)
        sbias = persist.tile([P, H, NLC], f32, tag="sbias")
        sb_scr = dramp.tile([H, L], f32, tag="sbscr")
        ones1 = persist.tile([1, P], bf16, tag="ones1")
        nc.vector.memset(ones1[:], 1.0)
        # ones rows at partitions 0 and 32 for row-broadcast matmuls
        ones_bank = persist.tile([33, P], f16, tag="onesbank")
        nc.vector.memset(ones_bank[:], 1.0)

        # packed row storage. Constraints: engine access base partition must
        # be 0/32/64, and dual-SBUF-input ops need equal input bases.
        # rowsF1 (f32): 0 sum_y (later rstd), 32 sum_y2
        # rowsF2 (f32): 0 u then var, 32 u^2
        # rowsF3 (f32): 0 mask row, 32 ln(var+eps)
        # rowsMM (f16): 0 u (matmul rhs), 32 rstd*m or rstd (matmul rhs)
        # sb_row (f32): PSUM->SBUF staging for the score-centering bias rows
        rowsF1 = persist.tile([33, L], f32, tag="rowsF1")
        rowsF2 = persist.tile([33, L], f32, tag="rowsF2")
        rowsF3 = persist.tile([33, L], f32, tag="rowsF3")
        rowsMM = persist.tile([33, L], f16, tag="rowsMM")
        sb_row = persist.tile([1, L], f32, tag="sbrow")
        nc.vector.memset(rowsF1[32:33, :], 0.0)
        nc.gpsimd.dma_start(
            out=rowsF3[0:1, :], in_=io["maskf"].rearrange("(a l) -> a l", a=1))

        # ---- big persistent tensors --------------------------------------
        if USE_FP8_SCORES:
            # [32*(h%4)+d%32, h//4, d//32, pos] per head h, head-channel d
            qT = persist.tile([P, 4, 2, L], f8, tag="qT")
            kT = persist.tile([P, 4, 2, L], f8, tag="kT")
        else:
            qT = persist.tile([P, NCH, L], bf16, tag="qT")
            kT = persist.tile([P, NCH, L], bf16, tag="kT")
        v_aug = persist.tile([P, NLC, H, P], f16, tag="vaug")
        y_sb = persist.tile([P, NCH, L], f16, tag="y")

        with tc.tile_pool(name="wts", bufs=1) as wts, \
             tc.tile_pool(name="mm", bufs=2, space="PSUM") as mmp, \
             tc.tile_pool(name="scp", bufs=2, space="PSUM") as scp, \
             tc.tile_pool(name="sbp", bufs=2, space="PSUM") as sbp, \
             tc.tile_pool(name="eT", bufs=3) as epool, \
             tc.tile_pool(name="att", bufs=2) as attp:

            seq_hi = wts.tile([P, NCH, L], f8, tag="seqh")
            seq_lo = wts.tile([P, NCH, L], f8, tag="seql")
            w_sb = {}
            for name in ("wq", "wk", "wv"):
                for half in ("hi", "lo"):
                    w_sb[name, half] = wts.tile([P, NCH, C], f8,
                                                tag=f"{name}{half}",
                                                name=f"{name}{half}")
            # DMA order = DMA_ENGINES serialization order: critical path
            # first. The prologue (Q/K chunks 0-1) needs seq hi+lo and W
            # cols 0:256 of wq/wk hi+lo; stream those, then remainders.
            B0 = 2 * P

            def w_dma(eng, name, half, lo, hi):
                eng.dma_start(
                    out=w_sb[name, half][:, :, lo:hi],
                    in_=io[f"{name}_{half}"][:, lo:hi].rearrange(
                        "(cc p) t -> p cc t", p=P))

            nc.sync.dma_start(
                out=seq_hi[:],
                in_=io["seq_hi"].rearrange("(cc p) l -> p cc l", p=P))
            w_dma(nc.scalar, "wk", "hi", 0, B0)
            nc.sync.dma_start(
                out=seq_lo[:],
                in_=io["seq_lo"].rearrange("(cc p) l -> p cc l", p=P))
            w_dma(nc.scalar, "wk", "lo", 0, B0)
            w_dma(nc.scalar, "wq", "hi", 0, B0)
            w_dma(nc.scalar, "wq", "lo", 0, B0)
            # wv cols 0:512 feed projv(0) in head slots 0-1: before the big
            # wq/wk remainders (needed only from slot 2 on)
            w_dma(nc.sync, "wv", "hi", 0, FD)
            w_dma(nc.sync, "wv", "lo", 0, FD)
            w_dma(nc.sync, "wk", "hi", B0, C)
            w_dma(nc.sync, "wk", "lo", B0, C)
            w_dma(nc.scalar, "wq", "hi", B0, C)
            w_dma(nc.scalar, "wq", "lo", B0, C)
            w_dma(nc.sync, "wv", "hi", FD, C)
            w_dma(nc.sync, "wv", "lo", FD, C)

            # ---- emission helpers ----------------------------------------
            PROJ_TERMS = (("hi", "hi"), ("lo", "hi"), ("hi", "lo"))

            def qk_lh(wname, dc, lh):
                """One [128, 512] projection tile of Q or K."""
                tgt = qT if wname == "wq" else kT
                b_sb = bq_sb if wname == "wq" else bk_sb
                xs = {"hi": seq_hi, "lo": seq_lo}
                ps = mmp.tile([P, FD], f32, tag="mm")
                n = len(PROJ_TERMS) * 4
                i = 0
                for wh, xh in PROJ_TERMS:
                    wt = w_sb[wname, wh]
                    xt = xs[xh]
                    for ccp in range(4):
                        nc.tensor.matmul(
                            ps[:],
                            lhsT=wt[:, 2 * ccp:2 * ccp + 2, dc * P:(dc + 1) * P],
                            rhs=xt[:, 2 * ccp:2 * ccp + 2, lh * FD:(lh + 1) * FD],
                            start=(i == 0), stop=(i == n - 1), perf_mode=DR)
                        i += 1
                if USE_FP8_SCORES:
                    # centered: max(z + b - CT, -CT) = relu(z + b) - CT
                    # (host pre-subtracts CT from the bias)
                    dst = tgt[:, dc // 2, dc % 2, lh * FD:(lh + 1) * FD]
                    lim = negc_col
                else:
                    dst = tgt[:, dc, lh * FD:(lh + 1) * FD]
                    lim = zero_col
                nc.vector.scalar_tensor_tensor(
                    out=dst, in0=ps[:], scalar=b_sb[:, dc:dc + 1],
                    in1=lim[:].to_broadcast((P, FD)),
                    op0=ALU.add, op1=ALU.max)

            def projv_lc(dh2, lc):
                """One [128 positions, 512 channels] V tile -> v_aug."""
                xs = {"hi": seq_hi, "lo": seq_lo}
                ps = mmp.tile([P, FD], f32, tag="mm")
                nc.tensor.matmul(
                    ps[:], lhsT=ones1[:], rhs=bv_row[:, dh2 * FD:(dh2 + 1) * FD],
                    start=True, stop=False)
                n = len(PROJ_TERMS) * 4
                i = 0
                for wh, xh in PROJ_TERMS:
                    wt = w_sb["wv", wh]
                    xt = xs[xh]
                    for ccp in range(4):
                        nc.tensor.matmul(
                            ps[:],
                            lhsT=xt[:, 2 * ccp:2 * ccp + 2, lc * P:(lc + 1) * P],
                            rhs=wt[:, 2 * ccp:2 * ccp + 2, dh2 * FD:(dh2 + 1) * FD],
                            start=False, stop=(i == n - 1), perf_mode=DR)
                        i += 1
                # relu + key-position mask: (ps max 0) * m[l]
                nc.vector.scalar_tensor_tensor(
                    out=v_aug[:, lc, dh2 * (H // 2):(dh2 + 1) * (H // 2), DH:P],
                    in0=ps[:], scalar=0.0,
                    in1=m_l[:, lc:lc + 1].to_broadcast((P, FD)),
                    op0=ALU.max, op1=ALU.mult)

            def av_qh(h, eT, qh):
                """Attention-value matmul + normalization for one query half."""
                ps = mmp.tile([P, FD], f32, tag="mm")
                for kc in range(NLC):
                    nc.tensor.matmul(
                        ps[:],
                        lhsT=v_aug[:, kc, h, :],
                        rhs=eT[:, kc, qh * FD:(qh + 1) * FD],
                        start=(kc == 0), stop=(kc == NLC - 1))
                rcp = attp.tile([DH, FD], f32, tag="rcp")
                nc.vector.reciprocal_approx_fast(out=rcp[:], in_=ps[0:DH, :])
                off = DH * (h % 2)
                nc.vector.tensor_mul(
                    y_sb[off:off + DH, h // 2, qh * FD:(qh + 1) * FD],
                    ps[DH:P, :], rcp[:])

            bg = deque()

            def bg_run(n):
                for _ in range(n):
                    if not bg:
                        return
                    bg.popleft()()

            eTs = {}

            def bias_prep(h):
                # per-key centering correction -> exp bias columns:
                # row = sum_d (CT*SCALE)*k~' over the head's channels, bounced
                # through DRAM to turn the key-row into per-chunk columns.
                g, r = h // 4, h % 4
                for half in range(NQ):
                    st = sbp.tile([1, FD], f32, tag="strow",
                                  name=f"sb_{h}_{half}")
                    nc.tensor.matmul(
                        st[:],
                        lhsT=cones[32 * r:32 * r + 32, :, 0:1],
                        rhs=kT[32 * r:32 * r + 32, g, 0:2,
                               half * FD:(half + 1) * FD],
                        start=True, stop=True, perf_mode=DR,
                        tile_position=(32 * r, 0))
                    nc.vector.tensor_copy(
                        sb_row[0:1, half * FD:(half + 1) * FD], st[:])
                    nc.sync.dma_start(
                        out=sb_scr[h, half * FD:(half + 1) * FD],
                        in_=sb_row[0:1, half * FD:(half + 1) * FD])
                    nc.sync.dma_start(
                        out=sbias[:, h, half * 4:(half + 1) * 4],
                        in_=sb_scr[h, half * FD:(half + 1) * FD].rearrange(
                            "(kc p) -> p kc", p=P))

            def scores_head(h):
                eT = epool.tile([P, NLC, L], f16, tag="eT", name=f"eT{h}")
                eTs[h] = eT
                g, r = h // 4, h % 4
                hp, off = h // 2, DH * (h % 2)
                for kc in range(NLC):
                    ps = scp.tile([P, L], f32, tag="sc")
                    for qh in range(NQ):
                        if USE_FP8_SCORES:
                            nc.tensor.matmul(
                                ps[:, qh * FD:(qh + 1) * FD],
                                lhsT=kT[32 * r:32 * r + 32, g, 0:2,
                                        kc * P:(kc + 1) * P],
                                rhs=qT[32 * r:32 * r + 32, g, 0:2,
                                       qh * FD:(qh + 1) * FD],
                                start=True, stop=True, perf_mode=DR,
                                tile_position=(32 * r, 0))
                        else:
                            nc.tensor.matmul(
                                ps[:, qh * FD:(qh + 1) * FD],
                                lhsT=kT[off:off + DH, hp, kc * P:(kc + 1) * P],
                                rhs=qT[off:off + DH, hp, qh * FD:(qh + 1) * FD],
                                start=True, stop=True)
                    if USE_FP8_SCORES:
                        nc.scalar.activation(eT[:, kc, :], ps[:], AF.Exp,
                                             scale=SCALE,
                                             bias=sbias[:, h, kc:kc + 1])
                    else:
                        nc.scalar.activation(eT[:, kc, :], ps[:], AF.Exp,
                                             scale=SCALE)
                    if kc >= 1:
                        bg_run(1)
                if USE_FP8_SCORES and h + 1 < H:
                    bias_prep(h + 1)

            def qk_items(dc):
                return [lambda w=w, dc=dc, lh=lh: qk_lh(w, dc, lh)
                        for w in ("wk", "wq") for lh in range(NQ)]

            def av_items(h):
                return [lambda h=h, qh=qh: av_qh(h, eTs[h], qh)
                        for qh in range(NQ)]

            # ---- prologue: Q/K chunks 0,1 feed the first head group ------
            for dc in (0, 1):
                for it in qk_items(dc):
                    it()
            if USE_FP8_SCORES:
                bias_prep(0)
            # mask columns 0:64 of v_aug (denominator rows of the AV matmul)
            ml_b = bass.AP(tensor=m_l.tensor, offset=m_l.offset,
                           ap=list(m_l[:].ap[:2]) + [[0, H], [0, DH]])
            nc.gpsimd.tensor_copy(v_aug[:, :, :, 0:DH], ml_b)

            # background enqueue plan, keyed by head slot
            plan = {
                0: [lambda lc=lc: projv_lc(0, lc) for lc in range(4)],
                1: [lambda lc=lc: projv_lc(0, lc) for lc in range(4, 8)],
                2: av_items(0) + qk_items(2),
                3: av_items(1) + qk_items(3),
                4: av_items(2),
                5: av_items(3) + qk_items(4),
                6: av_items(4) + qk_items(5),
                7: av_items(5)
                   + [lambda lc=lc: projv_lc(1, lc) for lc in range(4)],
                8: av_items(6)
                   + [lambda lc=lc: projv_lc(1, lc) for lc in range(4, 8)],
                9: av_items(7) + qk_items(6),
                10: av_items(8) + qk_items(7),
                11: av_items(9),
                12: av_items(10),
                13: av_items(11),
                14: av_items(12),
                15: av_items(13) + av_items(14),
            }
            for h in range(H):
                if h in plan:
                    bg.extend(plan[h])
                scores_head(h)
            while bg:
                bg.popleft()()
            for it in av_items(15):
                it()

        # ---- LayerNorm tail ----------------------------------------------
        # One-pass stats over y (the attention PSUM pools are closed, so the
        # accumulators get real PSUM banks): u = sum_y/C, var = sum_y2/C - u^2
        # (u^2 from the f32 u -- fp16 u would wreck var via ~100x cancellation)
        eps_col = persist.tile([1, 1], f32, tag="eps")
        nc.vector.memset(eps_col[:], EPS)
        with tc.tile_pool(name="styps", bufs=1, space="PSUM") as styps, \
             tc.tile_pool(name="sq", bufs=2) as sqp:
            ps_sy = styps.tile([1, L], f32, tag="sy")
            ps_sy2 = styps.tile([1, L], f32, tag="sy2")
            for cc in range(NCH):
                ysq = sqp.tile([P, L], f16, tag="ysq", name=f"ysq_{cc}")
                nc.vector.tensor_mul(ysq[:], y_sb[:, cc, :], y_sb[:, cc, :])
                for qh in range(NQ):
                    nc.tensor.matmul(
                        ps_sy[0:1, qh * FD:(qh + 1) * FD],
                        lhsT=ones_colh[:],
                        rhs=y_sb[:, cc, qh * FD:(qh + 1) * FD],
                        start=(cc == 0), stop=(cc == NCH - 1))
                    nc.tensor.matmul(
                        ps_sy2[0:1, qh * FD:(qh + 1) * FD],
                        lhsT=ones_colh[:],
                        rhs=ysq[:, qh * FD:(qh + 1) * FD],
                        start=(cc == 0), stop=(cc == NCH - 1))
            nc.vector.tensor_scalar_mul(rowsF2[0:1, :], ps_sy[0:1, :],
                                        1.0 / C)
            nc.vector.tensor_copy(rowsMM[0:1, :], rowsF2[0:1, :])
            nc.gpsimd.tensor_mul(rowsF2[32:33, :], rowsF2[0:1, :],
                                 rowsF2[0:1, :])
            nc.vector.scalar_tensor_tensor(
                out=rowsF2[0:1, :], in0=ps_sy2[0:1, :], scalar=1.0 / C,
                in1=rowsF2[32:33, :], op0=ALU.mult, op1=ALU.subtract)
        nc.scalar.activation(rowsF3[32:33, :], rowsF2[0:1, :], AF.Ln,
                             bias=eps_col[:, 0:1])
        nc.scalar.activation(rowsF1[0:1, :], rowsF3[32:33, :], AF.Exp,
                             scale=-0.5)

        with tc.tile_pool(name="tailps", bufs=1, space="PSUM") as tailps, \
             tc.tile_pool(name="norm", bufs=2) as norm:
            u_rep = tailps.tile([P, L], f32, tag="urep")
            for qh in range(NQ):
                nc.tensor.matmul(u_rep[:, qh * FD:(qh + 1) * FD],
                                 lhsT=ones_bank[0:1, :],
                                 rhs=rowsMM[0:1, qh * FD:(qh + 1) * FD],
                                 start=True, stop=True)
            u_rep_sb = norm.tile([P, L], f16, tag="ureps", bufs=1)
            nc.vector.tensor_copy(u_rep_sb[:], u_rep[:])

            out_r = out.rearrange("(cc p) l -> p cc l", p=P)
            if fast_ln:
                # ln_w == 1, ln_b == 0: out = (y - u) * (rstd * m), with the
                # f16 -> f32 widening done by the output DMA (SWDGE cast).
                nc.gpsimd.tensor_mul(rowsMM[32:33, :], rowsF1[0:1, :],
                                     rowsF3[0:1, :])
                rm_rep = tailps.tile([P, L], f32, tag="rmrep")
                for qh in range(NQ):
                    nc.tensor.matmul(rm_rep[:, qh * FD:(qh + 1) * FD],
                                     lhsT=ones_bank[32:33, :],
                                     rhs=rowsMM[32:33, qh * FD:(qh + 1) * FD],
                                     start=True, stop=True)
                rm_rep_sb = norm.tile([P, L], f16, tag="rmreps", bufs=1)
                nc.vector.tensor_copy(rm_rep_sb[:], rm_rep[:])
                for cc in range(NCH):
                    s = norm.tile([P, L], f16, tag="s", name=f"s_{cc}")
                    nc.vector.tensor_sub(s[:], y_sb[:, cc, :], u_rep_sb[:])
                    t3 = norm.tile([P, L], f16, tag="t3", name=f"t3_{cc}")
                    nc.vector.tensor_mul(t3[:], s[:], rm_rep_sb[:])
                    nc.gpsimd.dma_start(out=out_r[:, cc, :], in_=t3[:])
            else:
                nc.vector.tensor_copy(rowsMM[32:33, :], rowsF1[0:1, :])
                rm_rep = tailps.tile([P, L], f32, tag="rmrep")
                for qh in range(NQ):
                    nc.tensor.matmul(rm_rep[:, qh * FD:(qh + 1) * FD],
                                     lhsT=ones_bank[32:33, :],
                                     rhs=rowsMM[32:33, qh * FD:(qh + 1) * FD],
                                     start=True, stop=True)
                rm_rep_sb = norm.tile([P, L], f32, tag="rmrepsf", bufs=1)
                nc.vector.tensor_copy(rm_rep_sb[:], rm_rep[:])
                for cc in range(NCH):
                    t1 = norm.tile([P, L], f32, tag="t1", name=f"t1_{cc}")
                    nc.vector.tensor_sub(t1[:], y_sb[:, cc, :], u_rep_sb[:])
                    t2 = norm.tile([P, L], f32, tag="t2", name=f"t2_{cc}")
                    nc.vector.scalar_tensor_tensor(
                        out=t2[:], in0=t1[:], scalar=lnw_sb[:, cc:cc + 1],
                        in1=rm_rep_sb[:], op0=ALU.mult, op1=ALU.mult)
                    t3 = norm.tile([P, L], f32, tag="t3", name=f"t3_{cc}")
                    nc.vector.scalar_tensor_tensor(
                        out=t3[:], in0=t2[:], scalar=lnb_sb[:, cc:cc + 1],
                        in1=m_rep[:], op0=ALU.add, op1=ALU.mult)
                    dmae = nc.sync if cc % 2 == 0 else nc.scalar
                    dmae.dma_start(out=out_r[:, cc, :], in_=t3[:])


def _pin_act_table(nc):
    """Make every activation resolve to the one table that contains all the
    functions this kernel uses (Exp, Ln, Copy, Identity), so the compiler
    emits a single LoadActFuncSet."""
    from concourse.hw_specs import get_activation_tables
    keep = "natural_log_exp_and_others"
    try:
        tabs = get_activation_tables(nc.m.arch)
    except Exception:
        return
    if keep not in tabs:
        return
    shared = set(tabs[keep])
    for name, funcs in tabs.items():
        if name != keep:
            funcs -= shared


def build(fast_ln=True):
    if fast_ln in _BUILT:
        return _BUILT[fast_ln]
    nc = bacc.Bacc("TRN2", target_bir_lowering=False, debug=False,
                   num_devices=N_CORES)
    _pin_act_table(nc)
    io = {
        "seq_hi": nc.dram_tensor("seq_hi", [C, L], f8, kind="ExternalInput").ap(),
        "seq_lo": nc.dram_tensor("seq_lo", [C, L], f8, kind="ExternalInput").ap(),
        "maskf": nc.dram_tensor("maskf", [L], f32, kind="ExternalInput").ap(),
        "bq": nc.dram_tensor("bq", [C], f32, kind="ExternalInput").ap(),
        "bk": nc.dram_tensor("bk", [C], f32, kind="ExternalInput").ap(),
        "bv": nc.dram_tensor("bv", [C], bf16, kind="ExternalInput").ap(),
        "ln_w": nc.dram_tensor("ln_w", [C], f32, kind="ExternalInput").ap(),
        "ln_b": nc.dram_tensor("ln_b", [C], f32, kind="ExternalInput").ap(),
        "out": nc.dram_tensor("out", [C, L], f32, kind="ExternalOutput").ap(),
    }
    for name in ("wq", "wk", "wv"):
        for half in ("hi", "lo"):
            t = f"{name}_{half}"
            io[t] = nc.dram_tensor(t, [C, C], f8, kind="ExternalInput").ap()
    with tile.TileContext(nc) as tc:
        _emit(tc, io, fast_ln)
    nc.compile()
    _BUILT[fast_ln] = nc
    return nc


def _qk_perm():
    """Permutation of W rows so projection PSUM tiles land in the DoubleRow
    score layout: slot (tile tau, partition j) <- channel 64*h + d with
    h = 4*(tau//2) + j//32, d = 32*(tau%2) + j%32."""
    if not USE_FP8_SCORES:
        return np.arange(C)
    perm = np.empty(C, dtype=np.int64)
    for tau in range(NCH):
        for j in range(P):
            h = 4 * (tau // 2) + j // 32
            d = 32 * (tau % 2) + j % 32
            perm[tau * P + j] = 64 * h + d
    return perm


def _split_fp8(a):
    import ml_dtypes
    hi = a.astype(ml_dtypes.float8_e4m3)
    lo = (a - hi.astype(np.float32)).astype(ml_dtypes.float8_e4m3)
    return np.ascontiguousarray(hi), np.ascontiguousarray(lo)


def make_in_maps(seq, mask, wq, bq, wk, bk, wv, bv, ln_w, ln_b):
    import ml_dtypes
    seq = np.asarray(seq, dtype=np.float32)
    mask_f = np.ascontiguousarray(
        np.asarray(mask).astype(np.float32).reshape(N_CORES, L))
    perm = _qk_perm()
    wq_hi, wq_lo = _split_fp8(
        np.asarray(wq, np.float32)[perm, :].T * SW)
    wk_hi, wk_lo = _split_fp8(
        np.asarray(wk, np.float32)[perm, :].T * SW)
    wv_hi, wv_lo = _split_fp8(np.asarray(wv, np.float32).T * SW)
    shared = {
        "wq_hi": wq_hi, "wq_lo": wq_lo,
        "wk_hi": wk_hi, "wk_lo": wk_lo,
        "wv_hi": wv_hi, "wv_lo": wv_lo,
        "bq": np.ascontiguousarray(
            np.asarray(bq, np.float32)[perm] * SW
            - (CT if USE_FP8_SCORES else 0.0)),
        "bk": np.ascontiguousarray(
            np.asarray(bk, np.float32)[perm] * SW
            - (CT if USE_FP8_SCORES else 0.0)),
        "bv": np.ascontiguousarray(
            (np.asarray(bv, np.float32) * SW).astype(ml_dtypes.bfloat16)),
        "ln_w": np.ascontiguousarray(np.asarray(ln_w, dtype=np.float32)),
        "ln_b": np.ascontiguousarray(np.asarray(ln_b, dtype=np.float32)),
    }
    maps = []
    for i in range(N_CORES):
        s_hi, s_lo = _split_fp8(seq[i])
        maps.append({"seq_hi": s_hi, "seq_lo": s_lo, "maskf": mask_f[i],
                     **shared})
    return maps


def kernel(seq, mask, wq, bq, wk, bk, wv, bv, ln_w, ln_b):
    global LAST_RESULTS
    fast_ln = bool(np.all(np.asarray(ln_w) == 1.0)
                   and np.all(np.asarray(ln_b) == 0.0))
    nc = build(fast_ln)
    in_maps = make_in_maps(seq, mask, wq, bq, wk, bk, wv, bv, ln_w, ln_b)
    res = run_bass_kernel_spmd(nc, in_maps, list(range(N_CORES)))
    LAST_RESULTS = res
    return np.stack([res.results[i]["out"] for i in range(N_CORES)], axis=0)
